# revision 11
# baseline (speedup 1.0000x reference)
"""Trainium2 Bass kernel for MinimalLightningAttention2.

Strategy (8 NeuronCores, SPMD, no collectives):
  core c -> batch b = c // 4, head group g = c % 4 (heads 4g..4g+3).
  Each core computes, fully fused on-chip:
    qkv projection (its 4 heads' columns of Wqkv)
    chunked lightning-attention scan (L=128 chunks, per-head decay state S)
    row-parallel partial of the output projection (its 4 heads' rows of Wout)
  Host sums the 4 partial outputs per batch and adds bout.

Layouts on device (per core):
  xT   [c,   n]  via 2-byte transpose-DMA of host-cast bf16 x
  q,k  [d,   n]  (lhsT = Wq/Wk tile, rhs = xT)
  v    [n, h*d]  (lhsT = xT tile,    rhs = Wv)
  attn output oT [e, i] per head -> directly the lhsT of the Wout matmul.
All matmuls in bf16 (PSUM accumulation fp32); decay masks applied in fp32
during PSUM eviction.
"""

import math

import numpy as np
import ml_dtypes

B, N, C = 2, 4096, 2048
H_TOT = 16
HD = 128          # head dim
H = 4             # heads per core
L = 128           # attention chunk length
NCH = N // L      # 32 chunks
KT = C // 128     # 16 contraction tiles for the projections
NSPAN = 512       # tokens per outer iteration
NIT = N // NSPAN  # 8 outer iterations
P = 128

BF16 = ml_dtypes.bfloat16

_CACHE = {}


def _build():
    """Build + compile the SPMD Bass program (same program on all 8 cores)."""
    from contextlib import ExitStack

    import concourse.bass as bass
    import concourse.tile as tile
    from concourse import bacc, mybir

    DT = mybir.dt.bfloat16
    F32 = mybir.dt.float32

    nc = bacc.Bacc(
        "TRN2",
        target_bir_lowering=False,
        debug=False,
        enable_asserts=False,
        num_devices=8,
    )

    xd = nc.dram_tensor("x", [N, C], DT, kind="ExternalInput").ap()
    wqd = nc.dram_tensor("wq", [KT, P, H * HD], DT, kind="ExternalInput").ap()
    wkd = nc.dram_tensor("wk", [KT, P, H * HD], DT, kind="ExternalInput").ap()
    wvd = nc.dram_tensor("wv", [KT, P, H * HD], DT, kind="ExternalInput").ap()
    wod = nc.dram_tensor("wo", [H * HD, C], DT, kind="ExternalInput").ap()
    masktd = nc.dram_tensor("maskt", [P, H * L], F32, kind="ExternalInput").ap()
    qdecd = nc.dram_tensor("qdec", [P, H * NSPAN], F32, kind="ExternalInput").ap()
    kdecvd = nc.dram_tensor("kdecv", [P, H * HD], F32, kind="ExternalInput").ap()
    bdfd = nc.dram_tensor("bdf", [P, H * HD], F32, kind="ExternalInput").ap()
    bqkd = nc.dram_tensor("bqk", [P, 2 * H], F32, kind="ExternalInput").ap()
    bvfd = nc.dram_tensor("bvf", [P, H * HD], F32, kind="ExternalInput").ap()
    outd = nc.dram_tensor("out", [N, C], F32, kind="ExternalOutput").ap()

    mult = mybir.AluOpType.mult
    add = mybir.AluOpType.add

    with tile.TileContext(nc) as tc:
        with ExitStack() as ctx:
            const = ctx.enter_context(tc.tile_pool(name="const", bufs=1))
            xt_pool = ctx.enter_context(tc.tile_pool(name="xt", bufs=2))
            qk_pool = ctx.enter_context(tc.tile_pool(name="qk", bufs=2))
            sc_pool = ctx.enter_context(tc.tile_pool(name="sc", bufs=3))
            ob_pool = ctx.enter_context(tc.tile_pool(name="ob", bufs=3))
            outb_pool = ctx.enter_context(tc.tile_pool(name="outb", bufs=2))
            qkv_ps = ctx.enter_context(tc.tile_pool(name="qkvps", bufs=2, space="PSUM"))
            attn_ps = ctx.enter_context(tc.tile_pool(name="attnps", bufs=1, space="PSUM"))
            out_ps = ctx.enter_context(tc.tile_pool(name="outps", bufs=2, space="PSUM"))

            # ---- constants / weights resident in SBUF ----
            wq_sb = const.tile([P, KT * 512], DT)
            wk_sb = const.tile([P, KT * 512], DT)
            wv_sb = const.tile([P, KT * 512], DT)
            for kt in range(KT):
                nc.sync.dma_start(wq_sb[:, kt * 512:(kt + 1) * 512], wqd[kt])
                nc.sync.dma_start(wk_sb[:, kt * 512:(kt + 1) * 512], wkd[kt])
                nc.sync.dma_start(wv_sb[:, kt * 512:(kt + 1) * 512], wvd[kt])
            wo_sb = const.tile([P, H * C], DT)
            for h in range(H):
                nc.sync.dma_start(wo_sb[:, h * C:(h + 1) * C], wod[h * HD:(h + 1) * HD, :])
            maskt_sb = const.tile([P, H * L], F32)
            nc.sync.dma_start(maskt_sb[:], masktd[:])
            qdec_sb = const.tile([P, H * NSPAN], F32)
            nc.sync.dma_start(qdec_sb[:], qdecd[:])
            kdecv_sb = const.tile([P, H * HD], F32)
            nc.sync.dma_start(kdecv_sb[:], kdecvd[:])
            bdf_sb = const.tile([P, H * HD], F32)
            nc.sync.dma_start(bdf_sb[:], bdfd[:])
            bqk_sb = const.tile([P, 2 * H], F32)
            nc.sync.dma_start(bqk_sb[:], bqkd[:])
            bvf_sb = const.tile([P, H * HD], F32)
            nc.sync.dma_start(bvf_sb[:], bvfd[:])
            ident = const.tile([P, P], DT)
            from concourse.masks import make_identity
            make_identity(nc, ident)

            # per-head decay state S [d, e], 4 heads side by side, fp32
            S_sb = const.tile([P, H * HD], F32)
            nc.vector.memset(S_sb[:], 0.0)
            S_bf = const.tile([P, H * HD], DT)
            nc.vector.memset(S_bf[:], 0.0)

            for it in range(NIT):
                n0 = it * NSPAN
                # ---- load xT for this token span (transpose-DMA) ----
                xt = xt_pool.tile([P, KT * NSPAN], DT)
                for kt in range(KT):
                    nc.sync.dma_start(
                        xt[:, kt * NSPAN:(kt + 1) * NSPAN],
                        xd[n0:n0 + NSPAN, kt * P:(kt + 1) * P],
                        transpose=True,
                    )

                # ---- qkv projection for the span ----
                q_raw = qk_pool.tile([P, H * NSPAN], DT, tag="q_raw")
                q_dec = qk_pool.tile([P, H * NSPAN], DT, tag="q_dec")
                k_sb = qk_pool.tile([P, H * NSPAN], DT, tag="k_sb")
                v_sb = qk_pool.tile([P, H * NSPAN], DT, tag="v_sb")
                vdec = qk_pool.tile([P, H * NSPAN], DT, tag="vdec")

                for h in range(H):
                    ps = qkv_ps.tile([P, NSPAN], F32, tag="qkvps")
                    for kt in range(KT):
                        nc.tensor.matmul(
                            ps[:],
                            lhsT=wq_sb[:, kt * 512 + h * HD: kt * 512 + (h + 1) * HD],
                            rhs=xt[:, kt * NSPAN:(kt + 1) * NSPAN],
                            start=(kt == 0), stop=(kt == KT - 1),
                        )
                    # q_raw = psum + bq ; q_dec = (psum + bq) * qdec
                    nc.vector.tensor_scalar_add(q_raw[:, h * NSPAN:(h + 1) * NSPAN], ps[:], bqk_sb[:, 2 * h:2 * h + 1])
                    nc.vector.scalar_tensor_tensor(
                        q_dec[:, h * NSPAN:(h + 1) * NSPAN], ps[:], bqk_sb[:, 2 * h:2 * h + 1],
                        qdec_sb[:, h * NSPAN:(h + 1) * NSPAN], op0=add, op1=mult,
                    )
                    ps = qkv_ps.tile([P, NSPAN], F32, tag="qkvps")
                    for kt in range(KT):
                        nc.tensor.matmul(
                            ps[:],
                            lhsT=wk_sb[:, kt * 512 + h * HD: kt * 512 + (h + 1) * HD],
                            rhs=xt[:, kt * NSPAN:(kt + 1) * NSPAN],
                            start=(kt == 0), stop=(kt == KT - 1),
                        )
                    nc.scalar.activation(
                        k_sb[:, h * NSPAN:(h + 1) * NSPAN], ps[:],
                        mybir.ActivationFunctionType.Identity, bias=bqk_sb[:, 2 * h + 1:2 * h + 2],
                    )

                for ns in range(4):
                    ps = qkv_ps.tile([P, NSPAN], F32, tag="qkvps")
                    for kt in range(KT):
                        nc.tensor.matmul(
                            ps[:],
                            lhsT=xt[:, kt * NSPAN + ns * P: kt * NSPAN + (ns + 1) * P],
                            rhs=wv_sb[:, kt * 512:(kt + 1) * 512],
                            start=(kt == 0), stop=(kt == KT - 1),
                        )
                    nc.vector.tensor_tensor(v_sb[:, ns * 512:(ns + 1) * 512], ps[:], bvf_sb[:], op=add)
                    nc.vector.tensor_tensor(vdec[:, ns * 512:(ns + 1) * 512], v_sb[:, ns * 512:(ns + 1) * 512], kdecv_sb[:], op=mult)

                # ---- attention + output projection, chunk by chunk ----
                for p in range(4):
                    # scoresT for all 4 heads into one psum bank
                    sc_ps = attn_ps.tile([P, 512], F32, tag="sc")
                    for h in range(H):
                        nc.tensor.matmul(
                            sc_ps[:, h * L:(h + 1) * L],
                            lhsT=k_sb[:, h * NSPAN + p * L: h * NSPAN + (p + 1) * L],
                            rhs=q_raw[:, h * NSPAN + p * L: h * NSPAN + (p + 1) * L],
                            start=True, stop=True,
                        )
                    scT = sc_pool.tile([P, 512], DT, tag="scT")
                    nc.vector.tensor_tensor(scT[:], sc_ps[:], maskt_sb[:], op=mult)

                    # kT (transpose k chunk) for all 4 heads
                    kt_ps = attn_ps.tile([P, 512], DT, tag="ktp")
                    for h in range(H):
                        nc.tensor.transpose(
                            kt_ps[:, h * HD:(h + 1) * HD],
                            k_sb[:, h * NSPAN + p * L: h * NSPAN + (p + 1) * L],
                            ident[:],
                        )
                    kT = sc_pool.tile([P, 512], DT, tag="kT")
                    nc.scalar.copy(kT[:], kt_ps[:])

                    # o = v^T @ scoresT + S^T @ qdec   [e, i] per head
                    o_ps = attn_ps.tile([P, 512], F32, tag="o")
                    for h in range(H):
                        nc.tensor.matmul(
                            o_ps[:, h * L:(h + 1) * L],
                            lhsT=v_sb[:, p * 512 + h * HD: p * 512 + (h + 1) * HD],
                            rhs=scT[:, h * L:(h + 1) * L],
                            start=True, stop=False,
                        )
                        nc.tensor.matmul(
                            o_ps[:, h * L:(h + 1) * L],
                            lhsT=S_bf[:, h * HD:(h + 1) * HD],
                            rhs=q_dec[:, h * NSPAN + p * L: h * NSPAN + (p + 1) * L],
                            start=False, stop=True,
                        )
                    ob = ob_pool.tile([P, 512], DT, tag="ob")
                    nc.vector.tensor_copy(ob[:], o_ps[:])

                    # S <- S * blockdecay + kT^T @ vdec
                    su_ps = attn_ps.tile([P, 512], F32, tag="su")
                    for h in range(H):
                        nc.tensor.matmul(
                            su_ps[:, h * HD:(h + 1) * HD],
                            lhsT=kT[:, h * HD:(h + 1) * HD],
                            rhs=vdec[:, p * 512 + h * HD: p * 512 + (h + 1) * HD],
                            start=True, stop=True,
                        )
                    nc.vector.tensor_tensor(S_sb[:], S_sb[:], bdf_sb[:], op=mult)
                    nc.vector.tensor_tensor(S_sb[:], S_sb[:], su_ps[:], op=add)
                    nc.vector.tensor_copy(S_bf[:], S_sb[:])

                    # output projection for this chunk's 128 tokens
                    outb = outb_pool.tile([P, C], F32, tag="outb")
                    for ct in range(4):
                        ops = out_ps.tile([P, 512], F32, tag="outps")
                        for h in range(H):
                            nc.tensor.matmul(
                                ops[:],
                                lhsT=ob[:, h * L:(h + 1) * L],
                                rhs=wo_sb[:, h * C + ct * 512: h * C + (ct + 1) * 512],
                                start=(h == 0), stop=(h == H - 1),
                            )
                        if ct % 2 == 0:
                            nc.vector.tensor_copy(outb[:, ct * 512:(ct + 1) * 512], ops[:])
                        else:
                            nc.scalar.copy(outb[:, ct * 512:(ct + 1) * 512], ops[:])
                    nc.sync.dma_start(outd[n0 + p * L: n0 + (p + 1) * L, :], outb[:])

    nc.compile()
    return nc


def _host_inputs(x, Wqkv, bqkv, Wout, bout, slopes):
    """Per-core input maps (numpy, host-side sharding + packing)."""
    in_maps = []
    i = np.arange(L, dtype=np.float64)
    for core in range(8):
        b, g = core // 4, core % 4
        h0 = 4 * g
        hsel = slice(h0 * HD, (h0 + H) * HD)

        xb = np.ascontiguousarray(x[b]).astype(BF16)

        wq = np.ascontiguousarray(Wqkv[:, 0 * C:1 * C][:, hsel].reshape(KT, P, H * HD)).astype(BF16)
        wk = np.ascontiguousarray(Wqkv[:, 1 * C:2 * C][:, hsel].reshape(KT, P, H * HD)).astype(BF16)
        wv = np.ascontiguousarray(Wqkv[:, 2 * C:3 * C][:, hsel].reshape(KT, P, H * HD)).astype(BF16)
        wo = np.ascontiguousarray(Wout[hsel, :]).astype(BF16)

        s = slopes[h0:h0 + H].astype(np.float64)  # (4,)
        diffT = (i[None, :] - i[:, None])          # [j, i] = i - j
        maskt = np.concatenate(
            [np.where(diffT >= 0, np.exp(-s[h] * diffT), 0.0) for h in range(H)],
            axis=1,
        ).astype(np.float32)                       # [128, 4*128]
        qdec_l = [np.exp(-s[h] * i) for h in range(H)]        # each (L,)
        qdec = np.concatenate(
            [np.broadcast_to(np.tile(qdec_l[h], NSPAN // L)[None, :], (P, NSPAN)) for h in range(H)],
            axis=1,
        ).astype(np.float32)                       # [128, 4*512]
        kdecv = np.concatenate(
            [np.broadcast_to(np.exp(-s[h] * (L - i))[:, None], (P, HD)) for h in range(H)],
            axis=1,
        ).astype(np.float32)                       # [128, 4*128]
        bdf = np.concatenate(
            [np.full((P, HD), math.exp(-s[h] * L)) for h in range(H)], axis=1
        ).astype(np.float32)
        # per-head, per-partition(d) q/k biases: columns [bq_h0, bk_h0, bq_h1, ...]
        bq_heads = bqkv[0 * C:1 * C][hsel].reshape(H, HD)
        bk_heads = bqkv[1 * C:2 * C][hsel].reshape(H, HD)
        bqk = np.zeros((P, 2 * H), dtype=np.float32)
        for h in range(H):
            bqk[:, 2 * h] = bq_heads[h]
            bqk[:, 2 * h + 1] = bk_heads[h]
        bvf = np.broadcast_to(bqkv[2 * C:3 * C][hsel][None, :], (P, H * HD)).astype(np.float32)

        in_maps.append({
            "x": xb, "wq": wq, "wk": wk, "wv": wv, "wo": wo,
            "maskt": maskt, "qdec": qdec, "kdecv": kdecv, "bdf": bdf,
            "bqk": bqk, "bvf": np.ascontiguousarray(bvf),
        })
    return in_maps


def kernel(x, Wqkv, bqkv, Wout, bout, slopes, _want_trace=False):
    from concourse import bass_utils

    x = np.asarray(x, dtype=np.float32)
    Wqkv = np.asarray(Wqkv, dtype=np.float32)
    bqkv = np.asarray(bqkv, dtype=np.float32)
    Wout = np.asarray(Wout, dtype=np.float32)
    bout = np.asarray(bout, dtype=np.float32)
    slopes = np.asarray(slopes, dtype=np.float32)

    if "nc" not in _CACHE:
        _CACHE["nc"] = _build()
    nc = _CACHE["nc"]

    in_maps = _host_inputs(x, Wqkv, bqkv, Wout, bout, slopes)
    res = bass_utils.run_bass_kernel_spmd(
        nc, in_maps, core_ids=list(range(8)), trace=_want_trace,
    )
    out = np.zeros((B, N, C), dtype=np.float32)
    for core in range(8):
        out[core // 4] += res.results[core]["out"]
    out += bout[None, None, :]
    if _want_trace:
        _CACHE["last_result"] = res
    return out


# revision 48
# speedup vs baseline: 1.2513x; 1.2513x over previous
"""Trainium2 Bass kernel for MinimalLightningAttention2.

Strategy (8 NeuronCores, SPMD, no collectives):
  core c -> batch b = c // 4, head group g = c % 4 (heads 4g..4g+3).
  Each core computes, fully fused on-chip:
    qkv projection (its 4 heads' columns of Wqkv)
    chunked lightning-attention scan (L=128 chunks, per-head decay state S)
    row-parallel partial of the output projection (its 4 heads' rows of Wout)
  Host sums the 4 partial outputs per batch and adds bout.

Layouts on device (per core):
  xT   [c,   n]  via 2-byte transpose-DMA of host-cast bf16 x
  q,k  [d,   n]  (lhsT = Wq/Wk tile, rhs = xT)
  v    [n, h*d]  (lhsT = xT tile,    rhs = Wv)
  attn output oT [e, i] per head -> directly the lhsT of the Wout matmul.
All matmuls in bf16 (PSUM accumulation fp32); decay masks applied in fp32
during PSUM eviction.
"""

import math

import numpy as np
import ml_dtypes

B, N, C = 2, 4096, 2048
H_TOT = 16
HD = 128          # head dim
H = 4             # heads per core
L = 128           # attention chunk length
NCH = N // L      # 32 chunks
KT = C // 128     # 16 contraction tiles for the projections
NSPAN = 512       # tokens per outer iteration
NIT = N // NSPAN  # 8 outer iterations
P = 128

BF16 = ml_dtypes.bfloat16

_CACHE = {}


def _build():
    """Build + compile the SPMD Bass program (same program on all 8 cores)."""
    from contextlib import ExitStack

    import concourse.bass as bass
    import concourse.tile as tile
    from concourse import bacc, mybir

    DT = mybir.dt.bfloat16
    F32 = mybir.dt.float32

    nc = bacc.Bacc(
        "TRN2",
        target_bir_lowering=False,
        debug=False,
        enable_asserts=False,
        num_devices=8,
    )

    # host-packed transpose of x: xtp[c, kt, n] = x[n, kt*128 + c]
    xd = nc.dram_tensor("x", [P, KT, N], DT, kind="ExternalInput").ap()
    # host-packed: [c, kt*512 + col] (col = head*128 + d), fully contiguous rows
    wqd = nc.dram_tensor("wq", [P, KT * 512], DT, kind="ExternalInput").ap()
    wkd = nc.dram_tensor("wk", [P, KT * 512], DT, kind="ExternalInput").ap()
    wvd = nc.dram_tensor("wv", [P, KT * 512], DT, kind="ExternalInput").ap()
    # host-packed: [d, h*2048 + outc]
    wod = nc.dram_tensor("wo", [P, H * C], DT, kind="ExternalInput").ap()
    masktd = nc.dram_tensor("maskt", [P, H * L], F32, kind="ExternalInput").ap()
    qdecd = nc.dram_tensor("qdec", [P, H * NSPAN], F32, kind="ExternalInput").ap()
    kdecvd = nc.dram_tensor("kdecv", [P, H * HD], F32, kind="ExternalInput").ap()
    bdfd = nc.dram_tensor("bdf", [P, H * HD], F32, kind="ExternalInput").ap()
    bqkd = nc.dram_tensor("bqk", [P, 2 * H], F32, kind="ExternalInput").ap()
    bvfd = nc.dram_tensor("bvf", [P, H * HD], F32, kind="ExternalInput").ap()
    outd = nc.dram_tensor("out", [N, C], F32, kind="ExternalOutput").ap()

    mult = mybir.AluOpType.mult
    add = mybir.AluOpType.add

    with tile.TileContext(nc) as tc:
        with ExitStack() as ctx:
            const = ctx.enter_context(tc.tile_pool(name="const", bufs=1))
            xt_pool = ctx.enter_context(tc.tile_pool(name="xt", bufs=2))
            qk_pool = ctx.enter_context(tc.tile_pool(name="qk", bufs=2))
            sc_pool = ctx.enter_context(tc.tile_pool(name="sc", bufs=3))
            ob_pool = ctx.enter_context(tc.tile_pool(name="ob", bufs=3))
            outb_pool = ctx.enter_context(tc.tile_pool(name="outb", bufs=2))
            qkv_ps = ctx.enter_context(tc.tile_pool(name="qkvps", bufs=2, space="PSUM"))
            attn_ps = ctx.enter_context(tc.tile_pool(name="attnps", bufs=1, space="PSUM"))
            out_ps = ctx.enter_context(tc.tile_pool(name="outps", bufs=2, space="PSUM"))

            # ---- constants / weights resident in SBUF ----
            # First token-span load goes first so the PE can start ASAP;
            # weight loads split across the two DGE paths (sync HW / gpsimd SW).
            HK = KT // 2
            xt0a = xt_pool.tile([P, HK, NSPAN], DT, tag="xta")
            nc.sync.dma_start(xt0a[:], xd[:, 0:HK, 0:NSPAN])

            # All big startup loads on the ONE sync ring, in need-order: each
            # DGE ring is a FIFO and rings fair-share HBM at packet
            # granularity, so a second ring would steal bandwidth from the
            # critical first loads. Small decay/bias constants go on the
            # gpsimd (SWDGE) ring.
            wq_h = [const.tile([P, HK * 512], DT, tag="wqh0", name="wqh0"), None]
            nc.sync.dma_start(wq_h[0][:], wqd[:, 0:HK * 512])
            xt0b = xt_pool.tile([P, KT - HK, NSPAN], DT, tag="xtb")
            nc.sync.dma_start(xt0b[:], xd[:, HK:KT, 0:NSPAN])
            wq_h[1] = const.tile([P, (KT - HK) * 512], DT, tag="wqh1", name="wqh1")
            nc.sync.dma_start(wq_h[1][:], wqd[:, HK * 512:])
            wk_h = []
            for hh in range(2):
                t = const.tile([P, HK * 512], DT, tag=f"wkh{hh}", name=f"wkh{hh}")
                nc.sync.dma_start(t[:], wkd[:, hh * HK * 512:(hh + 1) * HK * 512])
                wk_h.append(t)
            wv_sb = const.tile([P, KT * 512], DT)
            nc.sync.dma_start(wv_sb[:], wvd[:])
            wo_sb = const.tile([P, H * C], DT)
            nc.sync.dma_start(wo_sb[:], wod[:])
            qdec_sb = const.tile([P, H * NSPAN], F32)
            nc.gpsimd.dma_start(qdec_sb[:], qdecd[:])
            bqk_sb = const.tile([P, 2 * H], F32)
            nc.gpsimd.dma_start(bqk_sb[:], bqkd[:])
            kdecv_sb = const.tile([P, H * HD], F32)
            nc.gpsimd.dma_start(kdecv_sb[:], kdecvd[:])
            bdf_sb = const.tile([P, H * HD], F32)
            nc.gpsimd.dma_start(bdf_sb[:], bdfd[:])
            bvf_sb = const.tile([P, H * HD], F32)
            nc.gpsimd.dma_start(bvf_sb[:], bvfd[:])
            maskt_sb = const.tile([P, H * L], F32)
            nc.gpsimd.dma_start(maskt_sb[:], masktd[:])
            ident = const.tile([P, P], DT)
            from concourse.masks import make_identity
            make_identity(nc, ident)

            # per-head decay state S [d, e], 4 heads side by side, fp32
            S_sb = const.tile([P, H * HD], F32)
            nc.vector.memset(S_sb[:], 0.0)
            S_bf = const.tile([P, H * HD], DT)
            nc.vector.memset(S_bf[:], 0.0)

            xt_tiles = [(xt0a, xt0b)]
            for it in range(NIT):
                n0 = it * NSPAN
                # prefetch next span's xT (host-packed, two halves)
                if it + 1 < NIT:
                    xa = xt_pool.tile([P, HK, NSPAN], DT, tag="xta")
                    nc.sync.dma_start(xa[:], xd[:, 0:HK, n0 + NSPAN:n0 + 2 * NSPAN])
                    xb = xt_pool.tile([P, KT - HK, NSPAN], DT, tag="xtb")
                    nc.sync.dma_start(xb[:], xd[:, HK:KT, n0 + NSPAN:n0 + 2 * NSPAN])
                    xt_tiles.append((xa, xb))
                xtab = xt_tiles[it]

                def xts(kt):
                    return xtab[kt // HK][:, kt % HK, :]

                # ---- qkv projection for the span ----
                q_raw = qk_pool.tile([P, H * NSPAN], DT, tag="q_raw")
                q_dec = qk_pool.tile([P, H * NSPAN], DT, tag="q_dec")
                k_sb = qk_pool.tile([P, H * NSPAN], DT, tag="k_sb")
                v_sb = qk_pool.tile([P, H * NSPAN], DT, tag="v_sb")
                vdec = qk_pool.tile([P, H * NSPAN], DT, tag="vdec")

                for h in range(H):
                    ps = qkv_ps.tile([P, NSPAN], F32, tag="qkvps")
                    for kt in range(KT):
                        nc.tensor.matmul(
                            ps[:],
                            lhsT=wq_h[kt // HK][:, (kt % HK) * 512 + h * HD: (kt % HK) * 512 + (h + 1) * HD],
                            rhs=xts(kt),
                            start=(kt == 0), stop=(kt == KT - 1),
                        )
                    # q_raw = psum + bq ; q_dec = (psum + bq) * qdec
                    nc.vector.tensor_scalar_add(q_raw[:, h * NSPAN:(h + 1) * NSPAN], ps[:], bqk_sb[:, 2 * h:2 * h + 1])
                    nc.vector.scalar_tensor_tensor(
                        q_dec[:, h * NSPAN:(h + 1) * NSPAN], ps[:], bqk_sb[:, 2 * h:2 * h + 1],
                        qdec_sb[:, h * NSPAN:(h + 1) * NSPAN], op0=add, op1=mult,
                    )
                    ps = qkv_ps.tile([P, NSPAN], F32, tag="qkvps")
                    for kt in range(KT):
                        nc.tensor.matmul(
                            ps[:],
                            lhsT=wk_h[kt // HK][:, (kt % HK) * 512 + h * HD: (kt % HK) * 512 + (h + 1) * HD],
                            rhs=xts(kt),
                            start=(kt == 0), stop=(kt == KT - 1),
                        )
                    nc.scalar.activation(
                        k_sb[:, h * NSPAN:(h + 1) * NSPAN], ps[:],
                        mybir.ActivationFunctionType.Identity, bias=bqk_sb[:, 2 * h + 1:2 * h + 2],
                    )

                for ns in range(4):
                    ps = qkv_ps.tile([P, NSPAN], F32, tag="qkvps")
                    for kt in range(KT):
                        nc.tensor.matmul(
                            ps[:],
                            lhsT=xts(kt)[:, ns * P:(ns + 1) * P],
                            rhs=wv_sb[:, kt * 512:(kt + 1) * 512],
                            start=(kt == 0), stop=(kt == KT - 1),
                        )
                    nc.vector.tensor_tensor(v_sb[:, ns * 512:(ns + 1) * 512], ps[:], bvf_sb[:], op=add)
                    nc.vector.tensor_tensor(vdec[:, ns * 512:(ns + 1) * 512], v_sb[:, ns * 512:(ns + 1) * 512], kdecv_sb[:], op=mult)


                # ---- attention + output projection, chunk by chunk ----
                for p in range(4):
                    # scoresT for all 4 heads into one psum bank
                    sc_ps = attn_ps.tile([P, 512], F32, tag="sc")
                    for h in range(H):
                        nc.tensor.matmul(
                            sc_ps[:, h * L:(h + 1) * L],
                            lhsT=k_sb[:, h * NSPAN + p * L: h * NSPAN + (p + 1) * L],
                            rhs=q_raw[:, h * NSPAN + p * L: h * NSPAN + (p + 1) * L],
                            start=True, stop=True,
                        )
                    scT = sc_pool.tile([P, 512], DT, tag="scT")
                    nc.vector.tensor_tensor(scT[:], sc_ps[:], maskt_sb[:], op=mult)

                    # kT (transpose k chunk) for all 4 heads
                    kt_ps = attn_ps.tile([P, 512], DT, tag="ktp")
                    for h in range(H):
                        nc.tensor.transpose(
                            kt_ps[:, h * HD:(h + 1) * HD],
                            k_sb[:, h * NSPAN + p * L: h * NSPAN + (p + 1) * L],
                            ident[:],
                        )
                    kT = sc_pool.tile([P, 512], DT, tag="kT")
                    nc.scalar.copy(kT[:], kt_ps[:])

                    # o = v^T @ scoresT + S^T @ qdec   [e, i] per head
                    o_ps = attn_ps.tile([P, 512], F32, tag="o")
                    for h in range(H):
                        nc.tensor.matmul(
                            o_ps[:, h * L:(h + 1) * L],
                            lhsT=v_sb[:, p * 512 + h * HD: p * 512 + (h + 1) * HD],
                            rhs=scT[:, h * L:(h + 1) * L],
                            start=True, stop=False,
                        )
                        nc.tensor.matmul(
                            o_ps[:, h * L:(h + 1) * L],
                            lhsT=S_bf[:, h * HD:(h + 1) * HD],
                            rhs=q_dec[:, h * NSPAN + p * L: h * NSPAN + (p + 1) * L],
                            start=False, stop=True,
                        )
                    ob = ob_pool.tile([P, 512], DT, tag="ob")
                    nc.vector.tensor_copy(ob[:], o_ps[:])

                    # S <- S * blockdecay + kT^T @ vdec
                    su_ps = attn_ps.tile([P, 512], F32, tag="su")
                    for h in range(H):
                        nc.tensor.matmul(
                            su_ps[:, h * HD:(h + 1) * HD],
                            lhsT=kT[:, h * HD:(h + 1) * HD],
                            rhs=vdec[:, p * 512 + h * HD: p * 512 + (h + 1) * HD],
                            start=True, stop=True,
                        )
                    nc.vector.tensor_tensor(S_sb[:], S_sb[:], bdf_sb[:], op=mult)
                    nc.vector.tensor_tensor(S_sb[:], S_sb[:], su_ps[:], op=add)
                    nc.vector.tensor_copy(S_bf[:], S_sb[:])

                    # output projection for this chunk's 128 tokens
                    outb = outb_pool.tile([P, C], F32, tag="outb")
                    for ct in range(4):
                        ops = out_ps.tile([P, 512], F32, tag="outps")
                        for h in range(H):
                            nc.tensor.matmul(
                                ops[:],
                                lhsT=ob[:, h * L:(h + 1) * L],
                                rhs=wo_sb[:, h * C + ct * 512: h * C + (ct + 1) * 512],
                                start=(h == 0), stop=(h == H - 1),
                            )
                        if ct % 2 == 0:
                            nc.vector.tensor_copy(outb[:, ct * 512:(ct + 1) * 512], ops[:])
                        else:
                            nc.scalar.copy(outb[:, ct * 512:(ct + 1) * 512], ops[:])
                    nc.gpsimd.dma_start(outd[n0 + p * L: n0 + (p + 1) * L, :], outb[:])

    nc.compile()
    return nc


def _host_inputs(x, Wqkv, bqkv, Wout, bout, slopes):
    """Per-core input maps (numpy, host-side sharding + packing)."""
    in_maps = []
    # packed transpose of x, shared by the 4 cores of each batch:
    # xtp[c, kt, n] = x[b, n, kt*128 + c]
    _xtp_cache = [
        np.ascontiguousarray(
            x[b].astype(BF16).reshape(N, KT, P).transpose(2, 1, 0)
        )
        for b in range(B)
    ]
    i = np.arange(L, dtype=np.float64)
    for core in range(8):
        b, g = core // 4, core % 4
        h0 = 4 * g
        hsel = slice(h0 * HD, (h0 + H) * HD)

        xb = _xtp_cache[b]

        def pack_w(Wslice):
            # (C, 512) -> [c_in_tile(128), kt*512 + col]
            return np.ascontiguousarray(
                Wslice.astype(BF16).reshape(KT, P, H * HD).transpose(1, 0, 2).reshape(P, KT * 512)
            )

        wq = pack_w(Wqkv[:, 0 * C:1 * C][:, hsel])
        wk = pack_w(Wqkv[:, 1 * C:2 * C][:, hsel])
        wv = pack_w(Wqkv[:, 2 * C:3 * C][:, hsel])
        # Wout rows for these heads: [d(128), h*2048 + outc]
        wo = np.ascontiguousarray(
            Wout[hsel, :].astype(BF16).reshape(H, HD, C).transpose(1, 0, 2).reshape(P, H * C)
        )

        s = slopes[h0:h0 + H].astype(np.float64)  # (4,)
        diffT = (i[None, :] - i[:, None])          # [j, i] = i - j
        maskt = np.concatenate(
            [np.where(diffT >= 0, np.exp(-s[h] * diffT), 0.0) for h in range(H)],
            axis=1,
        ).astype(np.float32)                       # [128, 4*128]
        qdec_l = [np.exp(-s[h] * i) for h in range(H)]        # each (L,)
        qdec = np.concatenate(
            [np.broadcast_to(np.tile(qdec_l[h], NSPAN // L)[None, :], (P, NSPAN)) for h in range(H)],
            axis=1,
        ).astype(np.float32)                       # [128, 4*512]
        kdecv = np.concatenate(
            [np.broadcast_to(np.exp(-s[h] * (L - i))[:, None], (P, HD)) for h in range(H)],
            axis=1,
        ).astype(np.float32)                       # [128, 4*128]
        bdf = np.concatenate(
            [np.full((P, HD), math.exp(-s[h] * L)) for h in range(H)], axis=1
        ).astype(np.float32)
        # per-head, per-partition(d) q/k biases: columns [bq_h0, bk_h0, bq_h1, ...]
        bq_heads = bqkv[0 * C:1 * C][hsel].reshape(H, HD)
        bk_heads = bqkv[1 * C:2 * C][hsel].reshape(H, HD)
        bqk = np.zeros((P, 2 * H), dtype=np.float32)
        for h in range(H):
            bqk[:, 2 * h] = bq_heads[h]
            bqk[:, 2 * h + 1] = bk_heads[h]
        bvf = np.broadcast_to(bqkv[2 * C:3 * C][hsel][None, :], (P, H * HD)).astype(np.float32)

        in_maps.append({
            "x": xb, "wq": wq, "wk": wk, "wv": wv, "wo": wo,
            "maskt": maskt, "qdec": qdec, "kdecv": kdecv, "bdf": bdf,
            "bqk": bqk, "bvf": np.ascontiguousarray(bvf),
        })
    return in_maps


def kernel(x, Wqkv, bqkv, Wout, bout, slopes, _want_trace=False):
    from concourse import bass_utils

    x = np.asarray(x, dtype=np.float32)
    Wqkv = np.asarray(Wqkv, dtype=np.float32)
    bqkv = np.asarray(bqkv, dtype=np.float32)
    Wout = np.asarray(Wout, dtype=np.float32)
    bout = np.asarray(bout, dtype=np.float32)
    slopes = np.asarray(slopes, dtype=np.float32)

    if "nc" not in _CACHE:
        _CACHE["nc"] = _build()
    nc = _CACHE["nc"]

    in_maps = _host_inputs(x, Wqkv, bqkv, Wout, bout, slopes)
    res = bass_utils.run_bass_kernel_spmd(
        nc, in_maps, core_ids=list(range(8)), trace=_want_trace,
    )
    out = np.zeros((B, N, C), dtype=np.float32)
    for core in range(8):
        out[core // 4] += res.results[core]["out"]
    out += bout[None, None, :]
    if _want_trace:
        _CACHE["last_result"] = res
    return out
